# revision 25
# baseline (speedup 1.0000x reference)
"""Causal multi-head attention block (GPT-style) on 8 TRN2 NeuronCores.

Sharding: core (b, g) = batch b in {0,1} x head-group g in {0..3} (4 heads of
dh=64 each). Megatron-style: each core computes q/k/v projections for its 256
channels, attention for its 4 heads, and a partial c_proj using its 256 rows of
W_proj. Host sums the 4 partial projections per batch (+ bias terms).

On-core dataflow (all matmuls in float32r):
  qT,kT = (W_qk stationary) @ xT        -> [512, 2048]  (q pre-scaled by 1/8)
  v     = (xT stationary) @ W_v         -> [2048, 4*65] (ones column appended)
  sT    = kT_tile.T @ qT_slice          -> scores transposed [j, i]
  u     = exp(sT)  (no max-subtraction: scores are O(3); causal tiles only,
                    diagonal boundary masked by a [128,128] triangle multiply,
                    fully-masked columns skipped by the av matmuls)
  av    = (v|1 stationary) @ u          -> [65, 512]: rows 0-63 unnormalized
                                           attn-out^T, row 64 = softmax sums
  aT    = av[0:64] * (1/av[64]) via gpsimd partition_broadcast
  out   = (aT stationary) @ W_proj_rows -> partial [2048, 1024]

Emission interleaves attention units for i-slice gi with the projection
psum-groups of slice gi+1 (and the c_proj units during the last slice) so the
PE always has matmul work while ACT grinds through the exps.
"""

import sys

try:
    import concourse  # noqa: F401
except ImportError:
    sys.path.insert(0, "/opt/trn_rl_repo")

from contextlib import ExitStack

import numpy as np

import concourse.tile as tile
from concourse import bacc, mybir
from concourse.bass_utils import run_bass_kernel_spmd

F32 = mybir.dt.float32
F32R = mybir.dt.float32r
EXP = mybir.ActivationFunctionType.Exp
MUL = mybir.AluOpType.mult
ADD = mybir.AluOpType.add

B, T, D = 2, 2048, 1024
HG, DH = 4, 64          # heads per core, head dim
CQK = 512               # q+k channels per core
CV = 256                # v channels per core
KT = D // 128           # contraction tiles of the projections
TS = 512                # t-slice width
NTS = T // TS
NT128 = T // 128
VW = HG * (DH + 1)      # 260: per-t-tile v row (4 heads x (64 v + 1 ones))


def interleave(primary, filler, back=0.65):
    """Merge filler among primary units, weighted toward the back where the
    ACT pipeline debt is largest."""
    if not filler:
        return list(primary)
    n = len(primary)
    out = []
    fi = 0
    for i, p in enumerate(primary):
        out.append(p)
        # cumulative filler fraction: quadratic ramp controlled by `back`
        x = (i + 1) / n
        want = ((1 - back) * x + back * x * x) * len(filler)
        while fi < len(filler) and fi + 1 <= want:
            out.append(filler[fi])
            fi += 1
    out.extend(filler[fi:])
    return out


def build():
    nc = bacc.Bacc(None)

    xT_in = nc.dram_tensor("xT", [NTS, KT, 128, TS], F32R, kind="ExternalInput")
    wqk_in = nc.dram_tensor("wqk", [KT, 128, CQK], F32R, kind="ExternalInput")
    wv_in = nc.dram_tensor("wv", [KT, 128, CV], F32R, kind="ExternalInput")
    wp_in = nc.dram_tensor("wp", [2, 128, D], F32R, kind="ExternalInput")
    bias_in = nc.dram_tensor("bqk", [128, 4], F32, kind="ExternalInput")
    mask_in = nc.dram_tensor("mask", [128, 512], F32R, kind="ExternalInput")
    out_dram = nc.dram_tensor("out", [NT128, 2, 128, TS], F32, kind="ExternalOutput")

    with ExitStack() as ctx:
        tc = ctx.enter_context(tile.TileContext(nc))

        const = ctx.enter_context(tc.tile_pool(name="const", bufs=1))
        big = ctx.enter_context(tc.tile_pool(name="big", bufs=1))
        upool = ctx.enter_context(tc.tile_pool(name="upool", bufs=8))
        atup = ctx.enter_context(tc.tile_pool(name="atup", bufs=4))
        rows = ctx.enter_context(tc.tile_pool(name="rows", bufs=2))
        rbcp = ctx.enter_context(tc.tile_pool(name="rbcp", bufs=3))
        outp = ctx.enter_context(tc.tile_pool(name="outp", bufs=6))
        xrp = ctx.enter_context(tc.tile_pool(name="xrp", bufs=2))
        wqkp = ctx.enter_context(tc.tile_pool(name="wqkp", bufs=1))
        wvp = ctx.enter_context(tc.tile_pool(name="wvp", bufs=1))

        ps_s = ctx.enter_context(tc.tile_pool(name="ps_s", bufs=2, space="PSUM"))
        ps_av = ctx.enter_context(tc.tile_pool(name="ps_av", bufs=2, space="PSUM"))
        ps_mm = ctx.enter_context(tc.tile_pool(name="ps_mm", bufs=2, space="PSUM"))

        # constants
        bias_sb = const.tile([128, 4], F32, tag="bias")
        nc.sync.dma_start(out=bias_sb[:], in_=bias_in[:])
        tri = const.tile([128, 512], F32R, tag="tri")  # [zeros|tri, zeros|tri]
        ones128 = const.tile([128, 16], F32, tag="ones128")
        nc.vector.memset(ones128[:], 1.0)

        # persistent intermediates (split per t-slice to avoid false deps)
        qkT = {(ct, ts): big.tile([128, TS], F32R, tag=f"qkT{ct}_{ts}",
                                  name=f"qkT{ct}_{ts}")
               for ct in range(4) for ts in range(NTS)}
        Vt = [big.tile([128, 4 * VW], F32R, tag=f"Vt{ts}", name=f"Vt{ts}")
              for ts in range(NTS)]
        aT = {(hp, gi): big.tile([128, TS], F32R, tag=f"aT{hp}_{gi}",
                                 name=f"aT{hp}_{gi}")
              for hp in range(2) for gi in range(NTS)}
        wp = [big.tile([128, D], F32R, tag=f"wp{c}", name=f"wp{c}")
              for c in range(2)]

        # weight + first-slice DMAs (interleaved so queue round-robin gets the
        # first accumulation operands in early)
        wqk, wv, xr = [], [], {}
        for k in range(KT):
            x1 = xrp.tile([128, TS], F32R, tag=f"xr{k}", name=f"xr_0_{k}")
            nc.sync.dma_start(out=x1[:], in_=xT_in[0][k])
            xr[0, k] = x1
            w1 = wqkp.tile([128, CQK], F32R, tag=f"wqk{k}", name=f"wqk{k}")
            nc.sync.dma_start(out=w1[:], in_=wqk_in[k])
            wqk.append(w1)
        for k in range(KT):
            w2 = wvp.tile([128, CV], F32R, tag=f"wv{k}", name=f"wv{k}")
            nc.sync.dma_start(out=w2[:], in_=wv_in[k])
            wv.append(w2)
        nc.sync.dma_start(out=tri[:], in_=mask_in[:])
        for c in range(2):
            nc.sync.dma_start(out=wp[c][:], in_=wp_in[c])

        def load_xr(ts):
            def unit():
                for k in range(KT):
                    x1 = xrp.tile([128, TS], F32R, tag=f"xr{k}",
                                  name=f"xr_{ts}_{k}")
                    nc.sync.dma_start(out=x1[:], in_=xT_in[ts][k])
                    xr[ts, k] = x1
            return unit

        def qk_group(ts, ct):
            def unit():
                ps = ps_mm.tile([128, TS], F32, tag="mm", name=f"qk_{ts}_{ct}")
                for k in range(KT):
                    nc.tensor.matmul(ps[:], wqk[k][:, ct * 128:(ct + 1) * 128],
                                     xr[ts, k][:], start=(k == 0),
                                     stop=(k == KT - 1))
                scale = 0.125 if ct < 2 else 1.0
                nc.vector.tensor_scalar(
                    qkT[ct, ts][:], ps[:],
                    scale, bias_sb[:, ct:ct + 1], op0=MUL, op1=ADD)
            return unit

        def v_group(ts, sub):
            def unit():
                ps = ps_mm.tile([128, CV], F32, tag="mm", name=f"v_{ts}_{sub}")
                for k in range(KT):
                    nc.tensor.matmul(ps[:], xr[ts, k][:, sub * 128:(sub + 1) * 128],
                                     wv[k][:], start=(k == 0), stop=(k == KT - 1))
                v3 = Vt[ts][:].rearrange("p (s h e) -> p s h e", h=HG, e=DH + 1)
                nc.vector.tensor_copy(
                    v3[:, sub, :, 0:DH],
                    ps[:].rearrange("p (h e) -> p h e", e=DH))
                if sub == 0:
                    nc.vector.tensor_copy(
                        v3[:, :, :, DH],
                        ones128[:].rearrange("p (s h) -> p s h", h=HG))
            return unit

        avs = {}

        def att_alloc(gi):
            def unit():
                for hp in range(2):
                    for sub in range(2):
                        avs[gi, hp, sub] = ps_av.tile(
                            [65, TS], F32, tag="av", name=f"av_{gi}_{hp}_{sub}")
            return unit

        utiles = {}

        def att_sc(gi, hp, jt):
            def unit():
                d = jt * 128 - gi * TS
                c0 = min(max(d, 0), 256)  # N >= 256 keeps f32r at full rate
                ss = ps_s.tile([128, 2 * TS], F32, tag="ss",
                               name=f"ss_{gi}_{hp}_{jt}")
                jts = jt // 4  # t-slice of this j-tile
                jo = (jt % 4) * 128
                for half in range(2):
                    p0 = half * 64
                    nc.tensor.matmul(
                        ss[:, half * TS + c0:(half + 1) * TS],
                        qkT[2 + hp, jts][p0:p0 + 64, jo:jo + 128],
                        qkT[hp, gi][p0:p0 + 64, c0:TS],
                        start=True, stop=True)
                u = upool.tile([128, 2 * TS], F32R, tag="u",
                               name=f"u_{gi}_{hp}_{jt}")
                utiles[gi, hp, jt] = u
                u3 = u[:].rearrange("p (h i) -> p h i", h=2)
                s3 = ss[:].rearrange("p (h i) -> p h i", h=2)
                if c0:
                    nc.scalar.activation(u3[:, :, c0:TS], s3[:, :, c0:TS], EXP)
                else:
                    nc.scalar.activation(u[:], ss[:], EXP)
                if d >= 0:
                    # mask [c0, d+128): zeros on [c0, d), triangle on [d, d+128)
                    mw = 128 + d - c0
                    t3 = tri[:].rearrange("p (h m) -> p h m", h=2)
                    nc.vector.tensor_tensor(
                        u3[:, :, c0:d + 128], u3[:, :, c0:d + 128],
                        t3[:, :, 256 - mw:256], op=MUL)
            return unit

        def att_av(gi, hp, jt):
            njt = 4 * (gi + 1)

            def unit():
                d = jt * 128 - gi * TS
                c0 = min(max(d, 0), 256)
                jts = jt // 4
                u = utiles[gi, hp, jt]
                first, last = jt == 0, jt == njt - 1
                for half in range(2):
                    h = 2 * hp + half
                    nc.tensor.matmul(
                        avs[gi, hp, half][:, c0:TS],
                        Vt[jts][:, (jt % 4) * VW + h * 65:(jt % 4) * VW + (h + 1) * 65],
                        u[:, half * TS + c0:half * TS + TS],
                        start=first, stop=last)
            return unit

        def att_norm(gi, hp, sub):
            def unit():
                av = avs[gi, hp, sub]
                atu = atup.tile([65, TS], F32R, tag="atu",
                                name=f"atu_{gi}_{hp}_{sub}")
                if gi == NTS - 1:
                    nc.scalar.copy(atu[:], av[:])
                else:
                    nc.vector.tensor_copy(atu[:], av[:])
                r = rows.tile([1, TS], F32, tag="r", name=f"r_{gi}_{hp}_{sub}")
                nc.vector.reciprocal(r[:], atu[64:65, :])
                rbc = rbcp.tile([64, TS], F32, tag="rbc",
                                name=f"rbc_{gi}_{hp}_{sub}")
                nc.gpsimd.partition_broadcast(rbc[:], r[:])
                mul_eng = nc.vector if gi == NTS - 1 else nc.gpsimd
                mul_eng.tensor_tensor(
                    aT[hp, gi][sub * 64:(sub + 1) * 64, :],
                    atu[0:64, :], rbc[:], op=MUL)
            return unit

        def proj_unit(tt, nt):
            gi = tt // 4

            def unit():
                ps = ps_mm.tile([128, TS], F32, tag="mm", name=f"pj_{tt}_{nt}")
                for c in range(2):
                    nc.tensor.matmul(
                        ps[:], aT[c, gi][:, (tt % 4) * 128:(tt % 4 + 1) * 128],
                        wp[c][:, nt * TS:(nt + 1) * TS],
                        start=(c == 0), stop=(c == 1))
                o = outp.tile([128, TS], F32, tag="o", name=f"o_{tt}_{nt}")
                if tt >= 12:
                    nc.scalar.copy(o[:], ps[:])
                else:
                    nc.vector.tensor_copy(o[:], ps[:])
                nc.sync.dma_start(out=out_dram[tt][nt], in_=o[:])
            return unit

        def qk_part(ts, ct, ks, ke, ps_box):
            def unit():
                if ks == 0:
                    ps_box[0] = ps_mm.tile([128, TS], F32, tag="mm",
                                           name=f"qk_{ts}_{ct}")
                ps = ps_box[0]
                for k in range(ks, ke):
                    nc.tensor.matmul(ps[:], wqk[k][:, ct * 128:(ct + 1) * 128],
                                     xr[ts, k][:], start=(k == 0),
                                     stop=(k == KT - 1))
                if ke == KT:
                    scale = 0.125 if ct < 2 else 1.0
                    nc.vector.tensor_scalar(
                        qkT[ct, ts][:], ps[:],
                        scale, bias_sb[:, ct:ct + 1], op0=MUL, op1=ADD)
            return unit

        def phase_a_units(ts):
            us = []
            if ts > 0:
                us.append(load_xr(ts))
                for ct in range(4):
                    us.append(qk_group(ts, ct))
            else:
                boxes = [[None] for _ in range(4)]
                h = KT // 2
                us += [qk_part(0, 0, 0, h, boxes[0]), qk_part(0, 1, 0, h, boxes[1]),
                       qk_part(0, 0, h, KT, boxes[0]), qk_part(0, 1, h, KT, boxes[1]),
                       qk_part(0, 2, 0, h, boxes[2]), qk_part(0, 3, 0, h, boxes[3]),
                       qk_part(0, 2, h, KT, boxes[2]), qk_part(0, 3, h, KT, boxes[3])]
            for sub in range(4):
                us.append(v_group(ts, sub))
            return us

        def attention_units(gi):
            njt = 4 * (gi + 1)
            us = [att_alloc(gi)]
            for hp in range(2):
                us.append(att_sc(gi, hp, 0))
                if njt > 1:
                    us.append(att_sc(gi, hp, 1))
                for jt in range(2, njt):
                    us.append(att_sc(gi, hp, jt))
                    us.append(att_av(gi, hp, jt - 2))
                us.append(att_av(gi, hp, njt - 2))
                us.append(att_av(gi, hp, njt - 1))
                for sub in range(2):
                    us.append(att_norm(gi, hp, sub))
            return us

        # phase A for slice 0 runs alone (nothing to overlap with yet)
        for u in phase_a_units(0):
            u()
        # attention(gi) interleaved with projection work of slice gi+1;
        # during the last slice interleave the earlier c_proj units.
        for gi in range(NTS):
            if gi < NTS - 1:
                filler = phase_a_units(gi + 1)
            else:
                filler = [proj_unit(tt, nt)
                          for tt in range(0, 12) for nt in range(2)]
            for u in interleave(attention_units(gi), filler):
                u()
        for tt in range(12, 16):
            for nt in range(2):
                proj_unit(tt, nt)()

    nc.finalize()
    return nc


_NC = None


def _get_nc():
    global _NC
    if _NC is None:
        _NC = build()
    return _NC


def _make_in_maps(x, W_attn, b_attn, W_proj):
    jj = np.arange(128, dtype=np.int64)[:, None]
    ii = np.arange(128, dtype=np.int64)[None, :]
    tri = (jj <= ii).astype(np.float32)
    zt = np.concatenate([np.zeros((128, 128), np.float32), tri], axis=1)
    mask = np.ascontiguousarray(np.concatenate([zt, zt], axis=1))

    shards = []
    for g in range(4):
        q_cols = W_attn[:, g * CV:(g + 1) * CV]
        k_cols = W_attn[:, D + g * CV:D + (g + 1) * CV]
        wqk = np.ascontiguousarray(
            np.concatenate([q_cols, k_cols], axis=1)).reshape(KT, 128, CQK)
        wv = np.ascontiguousarray(
            W_attn[:, 2 * D + g * CV:2 * D + (g + 1) * CV]).reshape(KT, 128, CV)
        wp = np.ascontiguousarray(
            W_proj[g * CV:(g + 1) * CV, :]).reshape(2, 128, D)
        bq = b_attn[g * CV:(g + 1) * CV] / 8.0
        bk = b_attn[D + g * CV:D + (g + 1) * CV]
        bqk = np.ascontiguousarray(
            np.concatenate([bq, bk]).reshape(4, 128).T).astype(np.float32)
        shards.append((wqk, wv, wp, bqk))

    in_maps = []
    for b in range(B):
        xT = np.ascontiguousarray(x[b].T).reshape(KT, 128, NTS, TS)
        xT = np.ascontiguousarray(xT.transpose(2, 0, 1, 3))  # [NTS, KT, 128, TS]
        for g in range(4):
            wqk, wv, wp, bqk = shards[g]
            in_maps.append({
                "xT": xT, "wqk": wqk, "wv": wv, "wp": wp,
                "bqk": bqk, "mask": mask,
            })
    return in_maps


def run(inputs, trace=False):
    x = np.asarray(inputs["x"], dtype=np.float32)
    W_attn = np.asarray(inputs["W_attn"], dtype=np.float32)
    b_attn = np.asarray(inputs["b_attn"], dtype=np.float32)
    W_proj = np.asarray(inputs["W_proj"], dtype=np.float32)
    b_proj = np.asarray(inputs["b_proj"], dtype=np.float32)

    nc = _get_nc()
    in_maps = _make_in_maps(x, W_attn, b_attn, W_proj)
    res = run_bass_kernel_spmd(nc, in_maps, list(range(8)), trace=trace)

    out = np.zeros((B, T, D), dtype=np.float32)
    for b in range(B):
        for g in range(4):
            o = res.results[b * 4 + g]["out"]       # [16, 2, 128, 512]
            out[b] += o.transpose(0, 2, 1, 3).reshape(T, D)
    # v-bias contributes a constant shift through the value path; b_proj too.
    const = b_attn[2 * D:3 * D] @ W_proj + b_proj
    out += const[None, None, :].astype(np.float32)
    return out, res


def kernel(**inputs):
    out, _ = run(inputs, trace=False)
    return out



# revision 32
# speedup vs baseline: 1.0225x; 1.0225x over previous
"""Causal multi-head attention block (GPT-style) on 8 TRN2 NeuronCores.

Sharding: core (b, g) = batch b in {0,1} x head-group g in {0..3} (4 heads of
dh=64 each). Megatron-style: each core computes q/k/v projections for its 256
channels, attention for its 4 heads, and a partial c_proj using its 256 rows of
W_proj. Host sums the 4 partial projections per batch (+ bias terms).

On-core dataflow (all matmuls in float32r):
  qT,kT = (W_qk stationary) @ xT        -> [512, 2048]  (q pre-scaled by 1/8)
  v     = (xT stationary) @ W_v         -> [2048, 4*65] (ones column appended)
  sT    = kT_tile.T @ qT_slice          -> scores transposed [j, i]
  u     = exp(sT)  (no max-subtraction: scores are O(3); causal tiles only,
                    diagonal boundary masked by a [128,128] triangle multiply,
                    fully-masked columns skipped by the av matmuls)
  av    = (v|1 stationary) @ u          -> [65, 512]: rows 0-63 unnormalized
                                           attn-out^T, row 64 = softmax sums
  aT    = av[0:64] * (1/av[64]) via gpsimd partition_broadcast
  out   = (aT stationary) @ W_proj_rows -> partial [2048, 1024]

Emission interleaves attention units for i-slice gi with the projection
psum-groups of slice gi+1 (and the c_proj units during the last slice) so the
PE always has matmul work while ACT grinds through the exps.
"""

import sys

try:
    import concourse  # noqa: F401
except ImportError:
    sys.path.insert(0, "/opt/trn_rl_repo")

from contextlib import ExitStack

import numpy as np

import concourse.tile as tile
from concourse import bacc, mybir
from concourse.bass_utils import run_bass_kernel_spmd

F32 = mybir.dt.float32
F32R = mybir.dt.float32r
EXP = mybir.ActivationFunctionType.Exp
MUL = mybir.AluOpType.mult
ADD = mybir.AluOpType.add

B, T, D = 2, 2048, 1024
HG, DH = 4, 64          # heads per core, head dim
CQK = 512               # q+k channels per core
CV = 256                # v channels per core
KT = D // 128           # contraction tiles of the projections
TS = 512                # t-slice width
NTS = T // TS
NT128 = T // 128
VW = HG * (DH + 1)      # 260: per-t-tile v row (4 heads x (64 v + 1 ones))


def interleave(primary, filler, back=0.45):
    """Merge filler among primary units, weighted toward the back where the
    ACT pipeline debt is largest."""
    if not filler:
        return list(primary)
    n = len(primary)
    out = []
    fi = 0
    for i, p in enumerate(primary):
        out.append(p)
        # cumulative filler fraction: quadratic ramp controlled by `back`
        x = (i + 1) / n
        want = ((1 - back) * x + back * x * x) * len(filler)
        while fi < len(filler) and fi + 1 <= want:
            out.append(filler[fi])
            fi += 1
    out.extend(filler[fi:])
    return out


def build():
    nc = bacc.Bacc(None)

    xT_in = nc.dram_tensor("xT", [NTS, KT, 128, TS], F32R, kind="ExternalInput")
    wqk_in = nc.dram_tensor("wqk", [KT, 128, CQK], F32R, kind="ExternalInput")
    wv_in = nc.dram_tensor("wv", [KT, 128, CV], F32R, kind="ExternalInput")
    wp_in = nc.dram_tensor("wp", [2, 128, D], F32R, kind="ExternalInput")
    bias_in = nc.dram_tensor("bqk", [128, 4], F32, kind="ExternalInput")
    mask_in = nc.dram_tensor("mask", [128, 512], F32R, kind="ExternalInput")
    out_dram = nc.dram_tensor("out", [NT128, 2, 128, TS], F32, kind="ExternalOutput")

    with ExitStack() as ctx:
        tc = ctx.enter_context(tile.TileContext(nc))

        const = ctx.enter_context(tc.tile_pool(name="const", bufs=1))
        big = ctx.enter_context(tc.tile_pool(name="big", bufs=1))
        upool = ctx.enter_context(tc.tile_pool(name="upool", bufs=9))
        atup = ctx.enter_context(tc.tile_pool(name="atup", bufs=4))
        rows = ctx.enter_context(tc.tile_pool(name="rows", bufs=2))
        rbcp = ctx.enter_context(tc.tile_pool(name="rbcp", bufs=3))
        outp = ctx.enter_context(tc.tile_pool(name="outp", bufs=6))
        xrp = ctx.enter_context(tc.tile_pool(name="xrp", bufs=2))
        wqkp = ctx.enter_context(tc.tile_pool(name="wqkp", bufs=1))
        wvp = ctx.enter_context(tc.tile_pool(name="wvp", bufs=1))

        ps_s = ctx.enter_context(tc.tile_pool(name="ps_s", bufs=2, space="PSUM"))
        ps_av = ctx.enter_context(tc.tile_pool(name="ps_av", bufs=2, space="PSUM"))
        ps_mm = ctx.enter_context(tc.tile_pool(name="ps_mm", bufs=2, space="PSUM"))

        # constants
        bias_sb = const.tile([128, 4], F32, tag="bias")
        nc.sync.dma_start(out=bias_sb[:], in_=bias_in[:])
        tri = const.tile([128, 512], F32R, tag="tri")  # [zeros|tri, zeros|tri]
        ones128 = const.tile([128, 16], F32, tag="ones128")
        nc.vector.memset(ones128[:], 1.0)

        # persistent intermediates (split per t-slice to avoid false deps)
        qkT = {(ct, ts): big.tile([128, TS], F32R, tag=f"qkT{ct}_{ts}",
                                  name=f"qkT{ct}_{ts}")
               for ct in range(4) for ts in range(NTS)}
        Vt = [big.tile([128, 4 * VW], F32R, tag=f"Vt{ts}", name=f"Vt{ts}")
              for ts in range(NTS)]
        aT = {(hp, gi): big.tile([128, TS], F32R, tag=f"aT{hp}_{gi}",
                                 name=f"aT{hp}_{gi}")
              for hp in range(2) for gi in range(NTS)}
        wp = [big.tile([128, D], F32R, tag=f"wp{c}", name=f"wp{c}")
              for c in range(2)]

        # weight + first-slice DMAs (interleaved so queue round-robin gets the
        # first accumulation operands in early)
        wqk, wv, xr = [], [], {}
        for k in range(KT):
            x1 = xrp.tile([128, TS], F32R, tag=f"xr{k}", name=f"xr_0_{k}")
            nc.sync.dma_start(out=x1[:], in_=xT_in[0][k])
            xr[0, k] = x1
            w1 = wqkp.tile([128, CQK], F32R, tag=f"wqk{k}", name=f"wqk{k}")
            nc.sync.dma_start(out=w1[:], in_=wqk_in[k])
            wqk.append(w1)
        for k in range(KT):
            w2 = wvp.tile([128, CV], F32R, tag=f"wv{k}", name=f"wv{k}")
            nc.sync.dma_start(out=w2[:], in_=wv_in[k])
            wv.append(w2)
        nc.sync.dma_start(out=tri[:], in_=mask_in[:])
        for c in range(2):
            nc.sync.dma_start(out=wp[c][:], in_=wp_in[c])

        def load_xr(ts):
            def unit():
                for k in range(KT):
                    x1 = xrp.tile([128, TS], F32R, tag=f"xr{k}",
                                  name=f"xr_{ts}_{k}")
                    nc.sync.dma_start(out=x1[:], in_=xT_in[ts][k])
                    xr[ts, k] = x1
            return unit

        def qk_group(ts, ct):
            def unit():
                ps = ps_mm.tile([128, TS], F32, tag="mm", name=f"qk_{ts}_{ct}")
                for k in range(KT):
                    nc.tensor.matmul(ps[:], wqk[k][:, ct * 128:(ct + 1) * 128],
                                     xr[ts, k][:], start=(k == 0),
                                     stop=(k == KT - 1))
                scale = 0.125 if ct < 2 else 1.0
                nc.scalar.activation(
                    qkT[ct, ts][:], ps[:],
                    mybir.ActivationFunctionType.Identity,
                    bias=bias_sb[:, ct:ct + 1], scale=scale)
            return unit

        def v_group(ts, sub):
            def unit():
                ps = ps_mm.tile([128, CV], F32, tag="mm", name=f"v_{ts}_{sub}")
                for k in range(KT):
                    nc.tensor.matmul(ps[:], xr[ts, k][:, sub * 128:(sub + 1) * 128],
                                     wv[k][:], start=(k == 0), stop=(k == KT - 1))
                v3 = Vt[ts][:].rearrange("p (s h e) -> p s h e", h=HG, e=DH + 1)
                nc.scalar.copy(
                    v3[:, sub, :, 0:DH],
                    ps[:].rearrange("p (h e) -> p h e", e=DH))
                if sub == 0:
                    nc.vector.tensor_copy(
                        v3[:, :, :, DH],
                        ones128[:].rearrange("p (s h) -> p s h", h=HG))
            return unit

        avs = {}

        def att_alloc(gi):
            def unit():
                for hp in range(2):
                    for sub in range(2):
                        avs[gi, hp, sub] = ps_av.tile(
                            [65, TS], F32, tag="av", name=f"av_{gi}_{hp}_{sub}")
            return unit

        utiles = {}

        def att_sc(gi, hp, jt):
            def unit():
                d = jt * 128 - gi * TS
                c0 = min(max(d, 0), 256)  # N >= 256 keeps f32r at full rate
                ss = ps_s.tile([128, 2 * TS], F32, tag="ss",
                               name=f"ss_{gi}_{hp}_{jt}")
                jts = jt // 4  # t-slice of this j-tile
                jo = (jt % 4) * 128
                for half in range(2):
                    p0 = half * 64
                    nc.tensor.matmul(
                        ss[:, half * TS + c0:(half + 1) * TS],
                        qkT[2 + hp, jts][p0:p0 + 64, jo:jo + 128],
                        qkT[hp, gi][p0:p0 + 64, c0:TS],
                        start=True, stop=True)
                u = upool.tile([128, 2 * TS], F32R, tag="u",
                               name=f"u_{gi}_{hp}_{jt}")
                utiles[gi, hp, jt] = u
                u3 = u[:].rearrange("p (h i) -> p h i", h=2)
                s3 = ss[:].rearrange("p (h i) -> p h i", h=2)
                if c0:
                    nc.scalar.activation(u3[:, :, c0:TS], s3[:, :, c0:TS], EXP)
                else:
                    nc.scalar.activation(u[:], ss[:], EXP)
                if d >= 0:
                    # mask [c0, d+128): zeros on [c0, d), triangle on [d, d+128)
                    mw = 128 + d - c0
                    t3 = tri[:].rearrange("p (h m) -> p h m", h=2)
                    nc.vector.tensor_tensor(
                        u3[:, :, c0:d + 128], u3[:, :, c0:d + 128],
                        t3[:, :, 256 - mw:256], op=MUL)
            return unit

        def att_av(gi, hp, jt):
            njt = 4 * (gi + 1)

            def unit():
                d = jt * 128 - gi * TS
                c0 = min(max(d, 0), 256)
                jts = jt // 4
                u = utiles[gi, hp, jt]
                first, last = jt == 0, jt == njt - 1
                for half in range(2):
                    h = 2 * hp + half
                    nc.tensor.matmul(
                        avs[gi, hp, half][:, c0:TS],
                        Vt[jts][:, (jt % 4) * VW + h * 65:(jt % 4) * VW + (h + 1) * 65],
                        u[:, half * TS + c0:half * TS + TS],
                        start=first, stop=last)
            return unit

        def att_norm(gi, hp, sub):
            def unit():
                av = avs[gi, hp, sub]
                atu = atup.tile([65, TS], F32R, tag="atu",
                                name=f"atu_{gi}_{hp}_{sub}")
                if gi == NTS - 1:
                    nc.scalar.copy(atu[:], av[:])
                else:
                    nc.vector.tensor_copy(atu[:], av[:])
                r = rows.tile([1, TS], F32, tag="r", name=f"r_{gi}_{hp}_{sub}")
                nc.vector.reciprocal(r[:], atu[64:65, :])
                rbc = rbcp.tile([64, TS], F32, tag="rbc",
                                name=f"rbc_{gi}_{hp}_{sub}")
                nc.gpsimd.partition_broadcast(rbc[:], r[:])
                nc.vector.tensor_tensor(
                    aT[hp, gi][sub * 64:(sub + 1) * 64, :],
                    atu[0:64, :], rbc[:], op=MUL)
            return unit

        def proj_unit(tt, nt):
            gi = tt // 4

            def unit():
                ps = ps_mm.tile([128, TS], F32, tag="mm", name=f"pj_{tt}_{nt}")
                for c in range(2):
                    nc.tensor.matmul(
                        ps[:], aT[c, gi][:, (tt % 4) * 128:(tt % 4 + 1) * 128],
                        wp[c][:, nt * TS:(nt + 1) * TS],
                        start=(c == 0), stop=(c == 1))
                o = outp.tile([128, TS], F32, tag="o", name=f"o_{tt}_{nt}")
                nc.vector.tensor_copy(o[:], ps[:])
                nc.sync.dma_start(out=out_dram[tt][nt], in_=o[:])
            return unit

        def qk_part(ts, ct, ks, ke, ps_box):
            def unit():
                if ks == 0:
                    ps_box[0] = ps_mm.tile([128, TS], F32, tag="mm",
                                           name=f"qk_{ts}_{ct}")
                ps = ps_box[0]
                for k in range(ks, ke):
                    nc.tensor.matmul(ps[:], wqk[k][:, ct * 128:(ct + 1) * 128],
                                     xr[ts, k][:], start=(k == 0),
                                     stop=(k == KT - 1))
                if ke == KT:
                    scale = 0.125 if ct < 2 else 1.0
                    nc.vector.tensor_scalar(
                        qkT[ct, ts][:], ps[:],
                        scale, bias_sb[:, ct:ct + 1], op0=MUL, op1=ADD)
            return unit

        def phase_a_units(ts):
            us = []
            if ts > 0:
                us.append(load_xr(ts))
                for ct in range(4):
                    us.append(qk_group(ts, ct))
            else:
                boxes = [[None] for _ in range(4)]
                h = KT // 2
                us += [qk_part(0, 0, 0, h, boxes[0]), qk_part(0, 1, 0, h, boxes[1]),
                       qk_part(0, 0, h, KT, boxes[0]), qk_part(0, 1, h, KT, boxes[1]),
                       qk_part(0, 2, 0, h, boxes[2]), qk_part(0, 3, 0, h, boxes[3]),
                       qk_part(0, 2, h, KT, boxes[2]), qk_part(0, 3, h, KT, boxes[3])]
            for sub in range(4):
                us.append(v_group(ts, sub))
            return us

        def attention_units(gi):
            njt = 4 * (gi + 1)
            us = [att_alloc(gi)]
            # one flat pipeline across both head-pairs: av units trail their
            # exp by `depth` slots even across the hp boundary, so the ACT
            # stream never drains mid-slice
            seq = [(hp, jt) for hp in range(2) for jt in range(njt)]
            depth = min(5, njt)
            for idx, (hp, jt) in enumerate(seq):
                us.append(att_sc(gi, hp, jt))
                if idx >= depth:
                    phq, pjt = seq[idx - depth]
                    us.append(att_av(gi, phq, pjt))
                    if pjt == njt - 1:
                        for sub in range(2):
                            us.append(att_norm(gi, phq, sub))
            for idx in range(len(seq) - depth, len(seq)):
                phq, pjt = seq[idx]
                us.append(att_av(gi, phq, pjt))
                if pjt == njt - 1:
                    for sub in range(2):
                        us.append(att_norm(gi, phq, sub))
            return us

        # phase A for slice 0 runs alone (nothing to overlap with yet)
        for u in phase_a_units(0):
            u()
        # attention(gi) interleaved with projection work of slice gi+1;
        # during the last slice interleave the earlier c_proj units.
        for gi in range(NTS):
            if gi < NTS - 1:
                filler = phase_a_units(gi + 1)
            else:
                filler = [proj_unit(tt, nt)
                          for tt in range(0, 12) for nt in range(2)]
            for u in interleave(attention_units(gi), filler):
                u()
        for tt in range(12, 16):
            for nt in range(2):
                proj_unit(tt, nt)()

    nc.finalize()
    return nc


_NC = None


def _get_nc():
    global _NC
    if _NC is None:
        _NC = build()
    return _NC


def _make_in_maps(x, W_attn, b_attn, W_proj):
    jj = np.arange(128, dtype=np.int64)[:, None]
    ii = np.arange(128, dtype=np.int64)[None, :]
    tri = (jj <= ii).astype(np.float32)
    zt = np.concatenate([np.zeros((128, 128), np.float32), tri], axis=1)
    mask = np.ascontiguousarray(np.concatenate([zt, zt], axis=1))

    shards = []
    for g in range(4):
        q_cols = W_attn[:, g * CV:(g + 1) * CV]
        k_cols = W_attn[:, D + g * CV:D + (g + 1) * CV]
        wqk = np.ascontiguousarray(
            np.concatenate([q_cols, k_cols], axis=1)).reshape(KT, 128, CQK)
        wv = np.ascontiguousarray(
            W_attn[:, 2 * D + g * CV:2 * D + (g + 1) * CV]).reshape(KT, 128, CV)
        wp = np.ascontiguousarray(
            W_proj[g * CV:(g + 1) * CV, :]).reshape(2, 128, D)
        bq = b_attn[g * CV:(g + 1) * CV] / 8.0
        bk = b_attn[D + g * CV:D + (g + 1) * CV]
        bqk = np.ascontiguousarray(
            np.concatenate([bq, bk]).reshape(4, 128).T).astype(np.float32)
        shards.append((wqk, wv, wp, bqk))

    in_maps = []
    for b in range(B):
        xT = np.ascontiguousarray(x[b].T).reshape(KT, 128, NTS, TS)
        xT = np.ascontiguousarray(xT.transpose(2, 0, 1, 3))  # [NTS, KT, 128, TS]
        for g in range(4):
            wqk, wv, wp, bqk = shards[g]
            in_maps.append({
                "xT": xT, "wqk": wqk, "wv": wv, "wp": wp,
                "bqk": bqk, "mask": mask,
            })
    return in_maps


def run(inputs, trace=False):
    x = np.asarray(inputs["x"], dtype=np.float32)
    W_attn = np.asarray(inputs["W_attn"], dtype=np.float32)
    b_attn = np.asarray(inputs["b_attn"], dtype=np.float32)
    W_proj = np.asarray(inputs["W_proj"], dtype=np.float32)
    b_proj = np.asarray(inputs["b_proj"], dtype=np.float32)

    nc = _get_nc()
    in_maps = _make_in_maps(x, W_attn, b_attn, W_proj)
    res = run_bass_kernel_spmd(nc, in_maps, list(range(8)), trace=trace)

    out = np.zeros((B, T, D), dtype=np.float32)
    for b in range(B):
        for g in range(4):
            o = res.results[b * 4 + g]["out"]       # [16, 2, 128, 512]
            out[b] += o.transpose(0, 2, 1, 3).reshape(T, D)
    # v-bias contributes a constant shift through the value path; b_proj too.
    const = b_attn[2 * D:3 * D] @ W_proj + b_proj
    out += const[None, None, :].astype(np.float32)
    return out, res


def kernel(**inputs):
    out, _ = run(inputs, trace=False)
    return out

